# revision 1
# baseline (speedup 1.0000x reference)
"""Trainium2 Bass kernel for a 2-layer de-stationary-attention transformer.

Model (per reference):
  L=2 layers of: x += DSAttn(x); x = LN1(x); x = LN2(x + FFN(x)); then
  final LN + output projection Wp.
  DSAttn: softmax(scale * (Q K^T * tau + delta)) V with per-batch tau,
  per-(batch, key) delta.

Shapes: B=16, S=512, D=1024, H=16 heads (dh=64), F=4096.

Sharding: data-parallel over batch across 8 NeuronCores (2 batches/core),
weights replicated. No collectives.

Per-core layout strategy (all on-chip between DRAM load and store):
  - Residual stream x kept FEATURE-major: 8 SBUF tiles [128 (d), 1024 (tok)].
    All matmuls contract over the partition dim, so projections consume x
    directly as the moving operand.
  - Scores are computed pre-transposed S^T[s, l] = K_slice^T Q_slice so the
    softmax denominator direction lands on the free dim of the AV matmul,
    and tau/delta fold into the ScalarE exp as scale/bias (both
    per-partition after the transpose).
  - V is produced token-major [tok, dout] (x as the stationary operand) with
    a ones-column appended per head, so each AV matmul also emits the
    softmax denominator row for free.
  - Per-token (free-dim) normalizers (softmax recip, LN mean/rstd) are
    broadcast across partitions with K=1 matmuls into PSUM.
  - LayerNorm column sums via ones-vector matmuls (partition reduction on
    the PE), then a 3-op normalize (2 DVE + 1 ACT with per-partition
    gain/bias).
  - Matmuls run in float32r (reduced-precision fp32, full PE rate at
    N>=256), accumulating in fp32 PSUM.

Weights are pre-transposed/pre-tiled on the host so every DMA is a large
contiguous read. Host also pre-transposes x to feature-major and folds the
1/sqrt(dh) softmax scale into tau/delta.
"""

import sys

if "/opt/trn_rl_repo" not in sys.path:
    sys.path.insert(0, "/opt/trn_rl_repo")

import numpy as np

import concourse.bass as bass
import concourse.bacc as bacc
import concourse.tile as tile
import concourse.mybir as mybir
from concourse import bass_utils

# Model dims
L, D, H, F = 2, 1024, 16, 4096
B, S = 16, 512
DH = D // H  # 64
NCORES = 8
BPC = B // NCORES  # batches per core
P = 128
NDT = D // P        # 8 d-tiles
NST = S // P        # 4 s-tiles per batch
NTOK = BPC * S      # 1024 tokens per core
NHP = H // 2        # 8 head pairs
FC = 512            # FFN f-chunk size
NFC = F // FC       # 8 chunks
VW = DH + 1         # 65: value width per head incl. ones column
EPS = 1e-5

F32 = mybir.dt.float32
MM = mybir.dt.float32r  # matmul operand dtype
AF = mybir.ActivationFunctionType
ALU = mybir.AluOpType

_CACHE: dict = {}
import os
KPART = os.environ.get("KPART", "full")  # full | attn | ffn | noln
KGELU = os.environ.get("KGELU", "gelu")  # CoreSim lacks Gelu; "id" to swap


def _build(reps: int):
    key = (reps, KPART, KGELU)
    if key in _CACHE:
        return _CACHE[key]

    nc = bacc.Bacc("TRN2", target_bir_lowering=False, debug=False,
                   num_devices=NCORES)

    # ---- DRAM tensors (per-core shapes) ----
    # matmul-feeding tensors are float32r (same bits as f32)
    x_d = nc.dram_tensor("x_fm", (BPC, D, S), MM, kind="ExternalInput")
    wq_d = nc.dram_tensor("wq_t", (L, NDT, D, P), MM, kind="ExternalInput")
    wk_d = nc.dram_tensor("wk_t", (L, NDT, D, P), MM, kind="ExternalInput")
    wv_d = nc.dram_tensor("wv_t", (L, D, D), MM, kind="ExternalInput")
    wo_d = nc.dram_tensor("wo_t", (L, NDT, D, P), MM, kind="ExternalInput")
    w1_d = nc.dram_tensor("w1_t", (L, NFC, NDT, P, FC), MM, kind="ExternalInput")
    w2_d = nc.dram_tensor("w2_t", (L, F, D), MM, kind="ExternalInput")
    wp_d = nc.dram_tensor("wp_t", (NDT, D, P), MM, kind="ExternalInput")
    bv_d = nc.dram_tensor("bv", (L, D), MM, kind="ExternalInput")

    bq_d = nc.dram_tensor("bq", (L, D), F32, kind="ExternalInput")
    bk_d = nc.dram_tensor("bk", (L, D), F32, kind="ExternalInput")
    bo_d = nc.dram_tensor("bo", (L, D), F32, kind="ExternalInput")
    b1_d = nc.dram_tensor("b1", (L, F), F32, kind="ExternalInput")
    b2_d = nc.dram_tensor("b2", (L, D), F32, kind="ExternalInput")
    g1_d = nc.dram_tensor("g1", (L, D), F32, kind="ExternalInput")
    be1_d = nc.dram_tensor("be1", (L, D), F32, kind="ExternalInput")
    g2_d = nc.dram_tensor("g2", (L, D), F32, kind="ExternalInput")
    be2_d = nc.dram_tensor("be2", (L, D), F32, kind="ExternalInput")
    gf_d = nc.dram_tensor("gf", (D,), F32, kind="ExternalInput")
    bf_d = nc.dram_tensor("bf", (D,), F32, kind="ExternalInput")
    bp_d = nc.dram_tensor("bp", (D,), F32, kind="ExternalInput")
    ident_d = nc.dram_tensor("ident", (P, P), MM, kind="ExternalInput")
    stau_d = nc.dram_tensor("sc_tau", (BPC, P), F32, kind="ExternalInput")
    sdel_d = nc.dram_tensor("sc_delta", (BPC, S), F32, kind="ExternalInput")

    out_d = nc.dram_tensor("out_fm", (BPC, D, S), F32, kind="ExternalOutput")

    with tile.TileContext(nc) as tc:
        _emit(nc, tc, reps, locals())

    nc.compile()
    _CACHE[key] = nc
    return nc


def _emit(nc, tc, reps, d):
    x_d, wq_d, wk_d, wv_d, wo_d, w1_d, w2_d, wp_d = (
        d["x_d"], d["wq_d"], d["wk_d"], d["wv_d"], d["wo_d"], d["w1_d"],
        d["w2_d"], d["wp_d"])
    bv_d, bq_d, bk_d, bo_d, b1_d, b2_d = (
        d["bv_d"], d["bq_d"], d["bk_d"], d["bo_d"], d["b1_d"], d["b2_d"])
    g1_d, be1_d, g2_d, be2_d, gf_d, bf_d, bp_d = (
        d["g1_d"], d["be1_d"], d["g2_d"], d["be2_d"], d["gf_d"], d["bf_d"],
        d["bp_d"])
    stau_d, sdel_d, out_d = d["stau_d"], d["sdel_d"], d["out_d"]
    ident_d = d["ident_d"]

    from contextlib import ExitStack
    ctx = ExitStack()
    # Static SBUF budget is ~192KB/partition; non-overlapping-lifetime
    # buffers share pool tags (o/y, v/h, qk/w1, out/tmp).
    singles = ctx.enter_context(tc.tile_pool(name="singles", bufs=1))
    xpool = ctx.enter_context(tc.tile_pool(name="xpool", bufs=1))
    vhpool = ctx.enter_context(tc.tile_pool(name="vhpool", bufs=1))
    oypool = ctx.enter_context(tc.tile_pool(name="oypool", bufs=1))
    qw1pool = ctx.enter_context(tc.tile_pool(name="qw1pool", bufs=5))
    wpool = ctx.enter_context(tc.tile_pool(name="wpool", bufs=8))
    epool = ctx.enter_context(tc.tile_pool(name="epool", bufs=8))
    tmppool = ctx.enter_context(tc.tile_pool(name="tmppool", bufs=4))
    rowpool = ctx.enter_context(tc.tile_pool(name="rowpool", bufs=5))
    psA = ctx.enter_context(tc.tile_pool(name="psA", bufs=4, space="PSUM"))
    psX = ctx.enter_context(tc.tile_pool(name="psX", bufs=4, space="PSUM"))

    # ---- constants / params (loaded once, outside the reps loop) ----
    # memset cannot write float32r; bounce constants through f32 + ACT copy
    ones_col_f = singles.tile([P, 1], F32)
    nc.vector.memset(ones_col_f, 1.0)
    ones_col = singles.tile([P, 1], MM)
    nc.scalar.activation(ones_col, ones_col_f, AF.Copy)
    ones_row_f = singles.tile([1, P], F32)
    nc.vector.memset(ones_row_f, 1.0)
    ones_row = singles.tile([1, P], MM)
    nc.scalar.activation(ones_row, ones_row_f, AF.Copy)
    onesH_f = singles.tile([P, H], F32)
    nc.vector.memset(onesH_f, 1.0)
    eps_row = singles.tile([1, 1], F32)
    nc.vector.memset(eps_row, EPS)
    ident = singles.tile([P, P], MM)
    nc.sync.dma_start(ident, ident_d.ap())

    def load_cols(dram_row, ncols):
        # [ncols*P] DRAM vector -> [P, ncols] SBUF (partition-major)
        t = singles.tile([P, ncols], dram_row.dtype,
                         name=f"prm_{dram_row.tensor.name}_{nc.next_id()}")
        nc.sync.dma_start(t, dram_row.rearrange("(t p) -> p t", p=P))
        return t

    bq_sb, bk_sb, bo_sb, b2_sb = [], [], [], []
    g1_sb, be1_sb, g2_sb, be2_sb, b1_sb = [], [], [], [], []
    bv_sb = singles.tile([1, L * D], MM)
    for l in range(L):
        bq_sb.append(load_cols(bq_d[l], NDT))
        bk_sb.append(load_cols(bk_d[l], NDT))
        bo_sb.append(load_cols(bo_d[l], NDT))
        b2_sb.append(load_cols(b2_d[l], NDT))
        g1_sb.append(load_cols(g1_d[l], NDT))
        be1_sb.append(load_cols(be1_d[l], NDT))
        g2_sb.append(load_cols(g2_d[l], NDT))
        be2_sb.append(load_cols(be2_d[l], NDT))
        b1_sb.append(load_cols(b1_d[l], F // P))
        nc.sync.dma_start(bv_sb[:, l * D:(l + 1) * D], bv_d[l][None, :])
    gf_sb = load_cols(gf_d.ap(), NDT)
    bf_sb = load_cols(bf_d.ap(), NDT)
    bp_sb = load_cols(bp_d.ap(), NDT)
    stau_sb = singles.tile([P, BPC], F32)
    nc.sync.dma_start(stau_sb, stau_d.ap().rearrange("b p -> p b"))
    sdel_sb = singles.tile([P, BPC * NST], F32)
    nc.sync.dma_start(sdel_sb.rearrange("p (b t) -> p b t", b=BPC),
                      sdel_d.ap().rearrange("b (t p) -> p b t", p=P))

    def body(_i=None):
        # ---- load x (feature-major) ----
        x_sb = []
        for dt in range(NDT):
            xt = xpool.tile([P, NTOK], MM, name=f"x_{dt}", tag=f"x_{dt}")
            for b in range(BPC):
                nc.sync.dma_start(
                    xt[:, b * S:(b + 1) * S],
                    x_d[b, dt * P:(dt + 1) * P, :])
            x_sb.append(xt)

        def ln(src, dst, g_t, be_t):
            """LayerNorm over d (partitions): src/dst are lists of 8 fm
            tiles; g_t/be_t are [P, NDT] per-partition param tiles.
            Stats for both batches first (PE colsums + short row chains),
            then per-tile normalize: PE re-streams x into PSUM with the
            negated mean added (identity matmul + K=1 ones x mean_n), so
            the per-tile cost is one DVE mul + one ACT affine."""
            rows_rs, rows_nm = [], []
            for b in range(BPC):
                cs = slice(b * S, (b + 1) * S)
                ps_s = psA.tile([1, S], F32, name="ps_s", tag="ps")
                for dt in range(NDT):
                    nc.tensor.matmul(ps_s, ones_col, src[dt][:, cs],
                                     start=(dt == 0), stop=(dt == NDT - 1))
                ps_q = psA.tile([1, S], F32, name="ps_q", tag="ps")
                for dt in range(NDT):
                    sq = tmppool.tile([P, S], MM, name="sq", tag="tmp")
                    nc.scalar.activation(sq, src[dt][:, cs], AF.Square)
                    nc.tensor.matmul(ps_q, ones_col, sq,
                                     start=(dt == 0), stop=(dt == NDT - 1))
                mean_n = rowpool.tile([1, S], MM, name="mean_n", tag="row")
                nc.vector.tensor_scalar(mean_n, ps_s, -1.0 / D, None, ALU.mult)
                var = rowpool.tile([1, S], F32, name="var", tag="row")
                nc.vector.tensor_scalar(var, ps_q, 1.0 / D, None, ALU.mult)
                m2 = rowpool.tile([1, S], F32, name="m2", tag="row")
                nc.vector.tensor_mul(m2, mean_n, mean_n)
                nc.vector.tensor_sub(var, var, m2)
                sd = rowpool.tile([1, S], F32, name="sd", tag="row")
                nc.scalar.activation(sd, var, AF.Sqrt, bias=eps_row)
                rs_r = rowpool.tile([1, S], MM, name="rs_r", tag="row")
                with nc.allow_low_precision(reason="f32r rows feed matmuls"):
                    nc.vector.reciprocal(rs_r, sd)
                rows_rs.append(rs_r)
                rows_nm.append(mean_n)
            for b in range(BPC):
                cs = slice(b * S, (b + 1) * S)
                pb_rs = psX.tile([P, S], F32, name="pb_rs", tag="px")
                nc.tensor.matmul(pb_rs, ones_row, rows_rs[b])
                rs_sb = tmppool.tile([P, S], F32, name="rs_sb", tag="tmp")
                nc.scalar.activation(rs_sb, pb_rs, AF.Copy)
                for dt in range(NDT):
                    pc = psX.tile([P, S], F32, name="pc", tag="px")
                    nc.tensor.matmul(pc, ident, src[dt][:, cs],
                                     start=True, stop=False)
                    nc.tensor.matmul(pc, ones_row, rows_nm[b],
                                     start=False, stop=True)
                    t1 = tmppool.tile([P, S], F32, name="t1", tag="tmp")
                    nc.vector.tensor_mul(t1, pc, rs_sb)
                    nc.scalar.activation(dst[dt][:, cs], t1, AF.Identity,
                                         scale=g_t[:, dt:dt + 1],
                                         bias=be_t[:, dt:dt + 1])

        def attn_phase(l):
            # ================= attention =================
            # ---- V (token-major, ones col per head) ----
            wv_sb = []
            for dt in range(NDT):
                wt = wpool.tile([P, D], MM, name=f"wv_{dt}", tag="w")
                nc.sync.dma_start(wt, wv_d[l, dt * P:(dt + 1) * P, :])
                wv_sb.append(wt)
            v_sb = []
            for tt in range(NDT):
                vt = vhpool.tile([P, H * VW], MM, name=f"v_{tt}", tag=f"vh_{tt}")
                nc.scalar.activation(
                    vt.rearrange("p (h e) -> p h e", e=VW)[:, :, DH:DH + 1],
                    onesH_f.rearrange("p (h e) -> p h e", e=1), AF.Copy)
                v_sb.append(vt)
            for tt in range(NDT):
                ts = slice(tt * P, (tt + 1) * P)
                for nh in range(2):
                    ps = psA.tile([P, S], F32, name="ps_v", tag="ps")
                    for dt in range(NDT):
                        nc.tensor.matmul(
                            ps, x_sb[dt][:, ts],
                            wv_sb[dt][:, nh * 512:(nh + 1) * 512],
                            start=(dt == 0), stop=False)
                    nc.tensor.matmul(
                        ps, ones_row[:, :P],
                        bv_sb[:, l * D + nh * 512: l * D + (nh + 1) * 512],
                        start=False, stop=True)
                    dstv = v_sb[tt][:, nh * 8 * VW:(nh + 1) * 8 * VW]
                    nc.scalar.activation(
                        dstv.rearrange("p (h e) -> p h e", e=VW)[:, :, 0:DH],
                        ps.rearrange("p (h e) -> p h e", e=DH),
                        AF.Copy)
            # ---- per head pair: Q, K, scores, softmax, AV ----
            # Software-pipelined with a one-stage skew: head i+1's
            # scores+exp are emitted before head i's AV/normalize, so the
            # in-order PE queue never stalls waiting on ACT(exp)/DVE(recip).
            o_sb = []
            pending = []

            def s2_flush():
                if pending:
                    pending.pop(0)()

            for hp in range(NHP):
                wq_p = wpool.tile([P, NDT, P], MM, name="wq_p", tag="w")
                nc.sync.dma_start(
                    wq_p, wq_d[l, hp].rearrange("(t p) m -> p t m", p=P))
                wk_p = wpool.tile([P, NDT, P], MM, name="wk_p", tag="w")
                nc.sync.dma_start(
                    wk_p, wk_d[l, hp].rearrange("(t p) m -> p t m", p=P))
                q_p = qw1pool.tile([P, NTOK], MM, name="q_p", tag="qw1")
                k_p = qw1pool.tile([P, NTOK], MM, name="k_p", tag="qw1")
                for b in range(BPC):
                    cs = slice(b * S, (b + 1) * S)
                    for wt, dst, bias in ((wq_p, q_p, bq_sb[l]),
                                          (wk_p, k_p, bk_sb[l])):
                        ps = psA.tile([P, S], F32, name="ps_qk", tag="ps")
                        for dt in range(NDT):
                            nc.tensor.matmul(ps, wt[:, dt, :],
                                             x_sb[dt][:, cs],
                                             start=(dt == 0),
                                             stop=(dt == NDT - 1))
                        nc.scalar.activation(dst[:, cs], ps, AF.Identity,
                                             bias=bias[:, hp:hp + 1])
                ot = oypool.tile([P, NTOK], MM, name=f"o_{hp}", tag=f"oy_{hp}")
                o_sb.append(ot)
                if KPART == "proj":
                    nc.scalar.activation(ot, q_p, AF.Copy)
                    continue
                for b in range(BPC):
                    cs = slice(b * S, (b + 1) * S)
                    for lh in range(2):
                        h = hp * 2 + lh
                        rb = lh * DH
                        rsl = slice(rb, rb + DH)
                        ets = []
                        for st in range(NST):
                            ps = psA.tile([P, S], F32, name="ps_sc", tag="ps")
                            nc.tensor.matmul(
                                ps,
                                k_p[rsl, b * S + st * P: b * S + (st + 1) * P],
                                q_p[rsl, cs])
                            et = epool.tile([P, S], MM, name="et", tag="e")
                            if KPART == "noexp":
                                nc.scalar.activation(et, ps, AF.Exp)
                            else:
                                nc.scalar.activation(
                                    et, ps, AF.Exp,
                                    scale=stau_sb[:, b:b + 1],
                                    bias=sdel_sb[:, b * NST + st:
                                                 b * NST + st + 1])
                            ets.append(et)

                        def s2(ets=ets, ot=ot, cs=cs, h=h, b=b, rsl=rsl):
                            po = psX.tile([VW, S], F32, name="po", tag="px")
                            for st in range(NST):
                                nc.tensor.matmul(
                                    po,
                                    v_sb[b * NST + st][:, h * VW:(h + 1) * VW],
                                    ets[st], start=(st == 0),
                                    stop=(st == NST - 1))
                            if KPART == "nosm":
                                nc.scalar.activation(ot[rsl, cs], po[:DH, :],
                                                     AF.Copy)
                                return
                            den_r = rowpool.tile([1, S], MM, name="den_r",
                                                 tag="row")
                            with nc.allow_low_precision(
                                    reason="f32r rows feed matmuls"):
                                nc.vector.reciprocal(den_r, po[DH:DH + 1, :])
                            pb = psX.tile([P, S], F32, name="pb_at", tag="px")
                            nc.tensor.matmul(pb[:DH, :], ones_row[:, :DH],
                                             den_r)
                            onum = tmppool.tile([P, S], F32, name="onum",
                                                tag="tmp")
                            nc.scalar.activation(onum[:DH, :], po[:DH, :],
                                                 AF.Copy)
                            nc.vector.tensor_mul(ot[rsl, cs], onum[:DH, :],
                                                 pb[:DH, :])

                        pending.append(s2)
                        if len(pending) > 1:
                            s2_flush()
            while pending:
                s2_flush()
            # ---- Wo projection + residual into x ----
            for dto in range(NDT):
                wo_p = wpool.tile([P, NDT, P], MM, name="wo_p", tag="w")
                nc.sync.dma_start(
                    wo_p, wo_d[l, dto].rearrange("(t p) m -> p t m", p=P))
                for b in range(BPC):
                    cs = slice(b * S, (b + 1) * S)
                    ps = psA.tile([P, S], F32, name="ps_wo", tag="ps")
                    for dt in range(NDT):
                        nc.tensor.matmul(ps, wo_p[:, dt, :], o_sb[dt][:, cs],
                                         start=(dt == 0), stop=(dt == NDT - 1))
                    t2 = tmppool.tile([P, S], F32, name="t2", tag="tmp")
                    nc.scalar.activation(t2, ps, AF.Identity,
                                         bias=bo_sb[l][:, dto:dto + 1])
                    nc.vector.tensor_add(x_sb[dto][:, cs], x_sb[dto][:, cs],
                                         t2)

        def ffn_phase(l):
            # ================= FFN =================
            y_sb = []
            for dt in range(NDT):
                yt = oypool.tile([P, NTOK], MM, name=f"y_{dt}", tag=f"oy_{dt}")
                y_sb.append(yt)
            # Software-pipelined chunks: chunk c+1's h-matmuls are emitted
            # before chunk c's y-matmuls (h tiles alternate tag groups) so
            # the PE never stalls on the gelu eviction tail.
            def h_block(fc):
                w1_sb = []
                for j in range(NDT // 2):
                    wt = qw1pool.tile([P, 2, FC], MM, name="w1c", tag="qw1")
                    nc.sync.dma_start(
                        wt, w1_d[l, fc, 2 * j:2 * j + 2].rearrange(
                            "d p f -> p d f"))
                    w1_sb.append(wt)
                h_sb = []
                for ft in range(FC // P):
                    ht = vhpool.tile([P, NTOK], MM, name="htile",
                                     tag=f"vh_{(fc % 2) * 4 + ft}")
                    h_sb.append(ht)
                    gfi = fc * (FC // P) + ft
                    for b in range(BPC):
                        cs = slice(b * S, (b + 1) * S)
                        ps = psA.tile([P, S], F32, name="ps_h", tag="ps")
                        for dt in range(NDT):
                            nc.tensor.matmul(
                                ps,
                                w1_sb[dt // 2][:, dt % 2,
                                               ft * P:(ft + 1) * P],
                                x_sb[dt][:, cs],
                                start=(dt == 0), stop=(dt == NDT - 1))
                        gelu_f = AF.Gelu if KGELU == "gelu" else AF.Identity
                        nc.scalar.activation(ht[:, cs], ps, gelu_f,
                                             bias=b1_sb[l][:, gfi:gfi + 1])
                return h_sb

            def y_block(fc, h_sb):
                w2_sb = []
                for i in range(FC // P):
                    wt = wpool.tile([P, D], MM, name="w2c", tag="w")
                    r0 = fc * FC + i * P
                    nc.sync.dma_start(wt, w2_d[l, r0:r0 + P, :])
                    w2_sb.append(wt)
                for dt in range(NDT):
                    for b in range(BPC):
                        cs = slice(b * S, (b + 1) * S)
                        ps = psA.tile([P, S], F32, name="ps_y", tag="ps")
                        for ft in range(FC // P):
                            nc.tensor.matmul(
                                ps, w2_sb[ft][:, dt * P:(dt + 1) * P],
                                h_sb[ft][:, cs],
                                start=(ft == 0), stop=(ft == FC // P - 1))
                        if fc == 0:
                            nc.scalar.activation(y_sb[dt][:, cs], ps,
                                                 AF.Identity,
                                                 bias=b2_sb[l][:, dt:dt + 1])
                        else:
                            nc.vector.tensor_add(y_sb[dt][:, cs], ps,
                                                 y_sb[dt][:, cs])

            prev = None
            for fc in range(NFC):
                h_now = h_block(fc)
                if prev is not None:
                    y_block(fc - 1, prev)
                prev = h_now
            y_block(NFC - 1, prev)
            # residual r2 = x_ln1 + y (into y); LN2(y) -> x
            for dt in range(NDT):
                nc.vector.tensor_add(y_sb[dt], y_sb[dt], x_sb[dt])
            if KPART == "full":
                ln(y_sb, x_sb, g2_sb[l], be2_sb[l])

        ATTN_LIKE = ("full", "attn", "noln", "proj", "noexp", "nosm")
        for l in range(L):
            if KPART in ATTN_LIKE:
                attn_phase(l)
            if KPART == "full":
                ln(x_sb, x_sb, g1_sb[l], be1_sb[l])
            if KPART in ("full", "ffn", "noln"):
                ffn_phase(l)

        # ---- final LN + Wp ----
        if KPART == "full":
            ln(x_sb, x_sb, gf_sb, bf_sb)
        for dto in range(NDT):
            wp_p = wpool.tile([P, NDT, P], MM, name="wp_p", tag="w")
            nc.sync.dma_start(wp_p,
                              wp_d[dto].rearrange("(t p) m -> p t m", p=P))
            for b in range(BPC):
                cs = slice(b * S, (b + 1) * S)
                ps = psA.tile([P, S], F32, name="ps_wp", tag="ps")
                for dt in range(NDT):
                    nc.tensor.matmul(ps, wp_p[:, dt, :], x_sb[dt][:, cs],
                                     start=(dt == 0), stop=(dt == NDT - 1))
                op = tmppool.tile([P, S], F32, name="outp", tag="tmp")
                nc.scalar.activation(op, ps, AF.Identity,
                                     bias=bp_sb[:, dto:dto + 1])
                nc.sync.dma_start(out_d[b, dto * P:(dto + 1) * P, :], op)

    if reps == 1:
        body()
    else:
        with tc.For_i(0, reps, 1) as i:
            body(i)
    ctx.close()


# ======================= host side =======================

def _prep_core_inputs(inputs):
    """Build the 8 per-core input maps (weights shared, x/tau/delta sharded)."""
    f = np.float32
    x = np.asarray(inputs["x"], f)
    tau = np.asarray(inputs["tau"], f)
    delta = np.asarray(inputs["delta"], f)
    scale = 1.0 / np.sqrt(np.float32(DH))

    shared = {}
    wq = np.asarray(inputs["Wq"], f)
    wk = np.asarray(inputs["Wk"], f)
    wv = np.asarray(inputs["Wv"], f)
    wo = np.asarray(inputs["Wo"], f)
    w1 = np.asarray(inputs["W1"], f)
    w2 = np.asarray(inputs["W2"], f)
    wp = np.asarray(inputs["Wp"], f)

    def col_tiled(wt):  # [L, D, D] (already [din, dout]) -> [L, NDT, D, P]
        return np.ascontiguousarray(
            wt.reshape(L, D, NDT, P).transpose(0, 2, 1, 3))

    wq_t = col_tiled(wq.transpose(0, 2, 1))
    wk_t = col_tiled(wk.transpose(0, 2, 1))
    wo_t = col_tiled(wo.transpose(0, 2, 1))
    wv_t = np.ascontiguousarray(wv.transpose(0, 2, 1))
    # W1 [L, F, D] -> W1^T [L, D, F] -> [L, NFC, NDT, P, FC]
    w1_tr = w1.transpose(0, 2, 1)
    w1_t = np.ascontiguousarray(
        w1_tr.reshape(L, NDT, P, NFC, FC).transpose(0, 3, 1, 2, 4))
    w2_t = np.ascontiguousarray(w2.transpose(0, 2, 1))  # [L, F, D]
    wp_t = np.ascontiguousarray(
        wp.transpose(1, 0).reshape(D, NDT, P).transpose(1, 0, 2))

    shared = {
        "wq_t": wq_t, "wk_t": wk_t, "wv_t": wv_t, "wo_t": wo_t,
        "w1_t": w1_t, "w2_t": w2_t, "wp_t": wp_t,
        "bq": np.asarray(inputs["bq"], f), "bk": np.asarray(inputs["bk"], f),
        "bv": np.asarray(inputs["bv"], f), "bo": np.asarray(inputs["bo"], f),
        "b1": np.asarray(inputs["b1"], f), "b2": np.asarray(inputs["b2"], f),
        "g1": np.asarray(inputs["g1"], f), "be1": np.asarray(inputs["be1"], f),
        "g2": np.asarray(inputs["g2"], f), "be2": np.asarray(inputs["be2"], f),
        "gf": np.asarray(inputs["gf"], f), "bf": np.asarray(inputs["bf"], f),
        "bp": np.asarray(inputs["bp"], f),
    }

    shared["ident"] = np.eye(P, dtype=f)

    in_maps = []
    for c in range(NCORES):
        bs = slice(c * BPC, (c + 1) * BPC)
        m = dict(shared)
        m["x_fm"] = np.ascontiguousarray(x[bs].transpose(0, 2, 1))
        m["sc_tau"] = np.tile(tau[bs] * scale, (1, P)).astype(f)
        m["sc_delta"] = (delta[bs] * scale).astype(f)
        in_maps.append(m)
    return in_maps


def run(inputs, reps=1):
    nc = _build(reps)
    in_maps = _prep_core_inputs(inputs)
    res = bass_utils.run_bass_kernel_spmd(nc, in_maps,
                                          core_ids=list(range(NCORES)))
    outs = [res.results[c]["out_fm"].transpose(0, 2, 1) for c in range(NCORES)]
    return np.ascontiguousarray(np.concatenate(outs, axis=0))


def kernel(**inputs) -> np.ndarray:
    return run(inputs, reps=1)



# revision 5
# speedup vs baseline: 1.0935x; 1.0935x over previous
"""Trainium2 Bass kernel for a 2-layer de-stationary-attention transformer.

Model (per reference):
  L=2 layers of: x += DSAttn(x); x = LN1(x); x = LN2(x + FFN(x)); then
  final LN + output projection Wp.
  DSAttn: softmax(scale * (Q K^T * tau + delta)) V with per-batch tau,
  per-(batch, key) delta.

Shapes: B=16, S=512, D=1024, H=16 heads (dh=64), F=4096.

Sharding: data-parallel over batch across 8 NeuronCores (2 batches/core),
weights replicated. No collectives.

Per-core layout strategy (all on-chip between DRAM load and store):
  - Residual stream x kept FEATURE-major: 8 SBUF tiles [128 (d), 1024 (tok)].
    All matmuls contract over the partition dim, so projections consume x
    directly as the moving operand.
  - Scores are computed pre-transposed S^T[s, l] = K_slice^T Q_slice so the
    softmax denominator direction lands on the free dim of the AV matmul,
    and tau/delta fold into the ScalarE exp as scale/bias (both
    per-partition after the transpose).
  - V is produced token-major [tok, dout] (x as the stationary operand) with
    a ones-column appended per head, so each AV matmul also emits the
    softmax denominator row for free.
  - Per-token (free-dim) normalizers (softmax recip, LN mean/rstd) are
    broadcast across partitions with K=1 matmuls into PSUM.
  - LayerNorm column sums via ones-vector matmuls (partition reduction on
    the PE), then a 3-op normalize (2 DVE + 1 ACT with per-partition
    gain/bias).
  - Matmuls run in float32r (reduced-precision fp32, full PE rate at
    N>=256), accumulating in fp32 PSUM.

Weights are pre-transposed/pre-tiled on the host so every DMA is a large
contiguous read. Host also pre-transposes x to feature-major and folds the
1/sqrt(dh) softmax scale into tau/delta.
"""

import sys

if "/opt/trn_rl_repo" not in sys.path:
    sys.path.insert(0, "/opt/trn_rl_repo")

import numpy as np

import concourse.bass as bass
import concourse.bacc as bacc
import concourse.tile as tile
import concourse.mybir as mybir
from concourse import bass_utils

# Model dims
L, D, H, F = 2, 1024, 16, 4096
B, S = 16, 512
DH = D // H  # 64
NCORES = 8
BPC = B // NCORES  # batches per core
P = 128
NDT = D // P        # 8 d-tiles
NST = S // P        # 4 s-tiles per batch
NTOK = BPC * S      # 1024 tokens per core
NHP = H // 2        # 8 head pairs
FC = 512            # FFN f-chunk size
NFC = F // FC       # 8 chunks
VW = DH + 1         # 65: value width per head incl. ones column
EPS = 1e-5

F32 = mybir.dt.float32
MM = mybir.dt.bfloat16  # matmul operand dtype
import ml_dtypes
MMNP = np.dtype(mybir.dt.np(MM))
AF = mybir.ActivationFunctionType
ALU = mybir.AluOpType

_CACHE: dict = {}
import os
KPART = os.environ.get("KPART", "full")  # full | attn | ffn | noln
KGELU = os.environ.get("KGELU", "gelu")  # CoreSim lacks Gelu; "id" to swap


def _build(reps: int):
    key = (reps, KPART, KGELU)
    if key in _CACHE:
        return _CACHE[key]

    nc = bacc.Bacc("TRN2", target_bir_lowering=False, debug=False,
                   num_devices=NCORES)

    # ---- DRAM tensors (per-core shapes) ----
    # matmul-feeding tensors are float32r (same bits as f32)
    x_d = nc.dram_tensor("x_fm", (BPC, D, S), MM, kind="ExternalInput")
    wq_d = nc.dram_tensor("wq_t", (L, NDT, D, P), MM, kind="ExternalInput")
    wk_d = nc.dram_tensor("wk_t", (L, NDT, D, P), MM, kind="ExternalInput")
    wv_d = nc.dram_tensor("wv_t", (L, D, D), MM, kind="ExternalInput")
    wo_d = nc.dram_tensor("wo_t", (L, NDT, D, P), MM, kind="ExternalInput")
    w1_d = nc.dram_tensor("w1_t", (L, NFC, NDT, P, FC), MM, kind="ExternalInput")
    w2_d = nc.dram_tensor("w2_t", (L, F, D), MM, kind="ExternalInput")
    wp_d = nc.dram_tensor("wp_t", (NDT, D, P), MM, kind="ExternalInput")
    bv_d = nc.dram_tensor("bv", (L, D), MM, kind="ExternalInput")

    bq_d = nc.dram_tensor("bq", (L, D), F32, kind="ExternalInput")
    bk_d = nc.dram_tensor("bk", (L, D), F32, kind="ExternalInput")
    bo_d = nc.dram_tensor("bo", (L, D), F32, kind="ExternalInput")
    b1_d = nc.dram_tensor("b1", (L, F), F32, kind="ExternalInput")
    b2_d = nc.dram_tensor("b2", (L, D), F32, kind="ExternalInput")
    g1_d = nc.dram_tensor("g1", (L, D), F32, kind="ExternalInput")
    be1_d = nc.dram_tensor("be1", (L, D), F32, kind="ExternalInput")
    g2_d = nc.dram_tensor("g2", (L, D), F32, kind="ExternalInput")
    be2_d = nc.dram_tensor("be2", (L, D), F32, kind="ExternalInput")
    gf_d = nc.dram_tensor("gf", (D,), F32, kind="ExternalInput")
    bf_d = nc.dram_tensor("bf", (D,), F32, kind="ExternalInput")
    bp_d = nc.dram_tensor("bp", (D,), F32, kind="ExternalInput")
    ident_d = nc.dram_tensor("ident", (P, P), MM, kind="ExternalInput")
    stau_d = nc.dram_tensor("sc_tau", (BPC, P), F32, kind="ExternalInput")
    sdel_d = nc.dram_tensor("sc_delta", (BPC, S), F32, kind="ExternalInput")

    out_d = nc.dram_tensor("out_fm", (BPC, D, S), F32, kind="ExternalOutput")

    with tile.TileContext(nc) as tc:
        _emit(nc, tc, reps, locals())

    nc.compile()
    _CACHE[key] = nc
    return nc


def _emit(nc, tc, reps, d):
    x_d, wq_d, wk_d, wv_d, wo_d, w1_d, w2_d, wp_d = (
        d["x_d"], d["wq_d"], d["wk_d"], d["wv_d"], d["wo_d"], d["w1_d"],
        d["w2_d"], d["wp_d"])
    bv_d, bq_d, bk_d, bo_d, b1_d, b2_d = (
        d["bv_d"], d["bq_d"], d["bk_d"], d["bo_d"], d["b1_d"], d["b2_d"])
    g1_d, be1_d, g2_d, be2_d, gf_d, bf_d, bp_d = (
        d["g1_d"], d["be1_d"], d["g2_d"], d["be2_d"], d["gf_d"], d["bf_d"],
        d["bp_d"])
    stau_d, sdel_d, out_d = d["stau_d"], d["sdel_d"], d["out_d"]
    ident_d = d["ident_d"]

    from contextlib import ExitStack
    ctx = ExitStack()
    # Static SBUF budget is ~192KB/partition; non-overlapping-lifetime
    # buffers share pool tags (o/y, v/h, qk/w1, out/tmp).
    singles = ctx.enter_context(tc.tile_pool(name="singles", bufs=1))
    xpool = ctx.enter_context(tc.tile_pool(name="xpool", bufs=1))
    vhpool = ctx.enter_context(tc.tile_pool(name="vhpool", bufs=1))
    oypool = ctx.enter_context(tc.tile_pool(name="oypool", bufs=1))
    qw1pool = ctx.enter_context(tc.tile_pool(name="qw1pool", bufs=5))
    wpool = ctx.enter_context(tc.tile_pool(name="wpool", bufs=8))
    epool = ctx.enter_context(tc.tile_pool(name="epool", bufs=8))
    tmppool = ctx.enter_context(tc.tile_pool(name="tmppool", bufs=4))
    rowpool = ctx.enter_context(tc.tile_pool(name="rowpool", bufs=5))
    psA = ctx.enter_context(tc.tile_pool(name="psA", bufs=4, space="PSUM"))
    psX = ctx.enter_context(tc.tile_pool(name="psX", bufs=4, space="PSUM"))

    # ---- constants / params (loaded once, outside the reps loop) ----
    # memset cannot write float32r; bounce constants through f32 + ACT copy
    ones_col_f = singles.tile([P, 1], F32)
    nc.vector.memset(ones_col_f, 1.0)
    ones_col = singles.tile([P, 1], MM)
    nc.scalar.activation(ones_col, ones_col_f, AF.Copy)
    ones_row_f = singles.tile([1, P], F32)
    nc.vector.memset(ones_row_f, 1.0)
    ones_row = singles.tile([1, P], MM)
    nc.scalar.activation(ones_row, ones_row_f, AF.Copy)
    onesH_f = singles.tile([P, H], F32)
    nc.vector.memset(onesH_f, 1.0)
    eps_row = singles.tile([1, 1], F32)
    nc.vector.memset(eps_row, EPS)
    ident = singles.tile([P, P], MM)
    nc.sync.dma_start(ident, ident_d.ap())

    def load_cols(dram_row, ncols):
        # [ncols*P] DRAM vector -> [P, ncols] SBUF (partition-major)
        t = singles.tile([P, ncols], dram_row.dtype,
                         name=f"prm_{dram_row.tensor.name}_{nc.next_id()}")
        nc.sync.dma_start(t, dram_row.rearrange("(t p) -> p t", p=P))
        return t

    bq_sb, bk_sb, bo_sb, b2_sb = [], [], [], []
    g1_sb, be1_sb, g2_sb, be2_sb, b1_sb = [], [], [], [], []
    bv_sb = singles.tile([1, L * D], MM)
    for l in range(L):
        bq_sb.append(load_cols(bq_d[l], NDT))
        bk_sb.append(load_cols(bk_d[l], NDT))
        bo_sb.append(load_cols(bo_d[l], NDT))
        b2_sb.append(load_cols(b2_d[l], NDT))
        g1_sb.append(load_cols(g1_d[l], NDT))
        be1_sb.append(load_cols(be1_d[l], NDT))
        g2_sb.append(load_cols(g2_d[l], NDT))
        be2_sb.append(load_cols(be2_d[l], NDT))
        b1_sb.append(load_cols(b1_d[l], F // P))
        nc.sync.dma_start(bv_sb[:, l * D:(l + 1) * D], bv_d[l][None, :])
    gf_sb = load_cols(gf_d.ap(), NDT)
    bf_sb = load_cols(bf_d.ap(), NDT)
    bp_sb = load_cols(bp_d.ap(), NDT)
    stau_sb = singles.tile([P, BPC], F32)
    nc.sync.dma_start(stau_sb, stau_d.ap().rearrange("b p -> p b"))
    sdel_sb = singles.tile([P, BPC * NST], F32)
    nc.sync.dma_start(sdel_sb.rearrange("p (b t) -> p b t", b=BPC),
                      sdel_d.ap().rearrange("b (t p) -> p b t", p=P))

    def body(_i=None):
        # ---- load x (feature-major) ----
        x_sb = []
        for dt in range(NDT):
            xt = xpool.tile([P, NTOK], MM, name=f"x_{dt}", tag=f"x_{dt}")
            for b in range(BPC):
                nc.sync.dma_start(
                    xt[:, b * S:(b + 1) * S],
                    x_d[b, dt * P:(dt + 1) * P, :])
            x_sb.append(xt)

        def ln(src, dst, g_t, be_t):
            """LayerNorm over d (partitions): src/dst are lists of 8 fm
            tiles; g_t/be_t are [P, NDT] per-partition param tiles.
            Stats for both batches first (PE colsums + short row chains),
            then per-tile normalize: PE re-streams x into PSUM with the
            negated mean added (identity matmul + K=1 ones x mean_n), so
            the per-tile cost is one DVE mul + one ACT affine."""
            rows_rs, rows_nm = [], []
            for b in range(BPC):
                cs = slice(b * S, (b + 1) * S)
                ps_s = psA.tile([1, S], F32, name="ps_s", tag="ps")
                for dt in range(NDT):
                    nc.tensor.matmul(ps_s, ones_col, src[dt][:, cs],
                                     start=(dt == 0), stop=(dt == NDT - 1))
                ps_q = psA.tile([1, S], F32, name="ps_q", tag="ps")
                for dt in range(NDT):
                    sq = tmppool.tile([P, S], MM, name="sq", tag="tmp")
                    nc.scalar.activation(sq, src[dt][:, cs], AF.Square)
                    nc.tensor.matmul(ps_q, ones_col, sq,
                                     start=(dt == 0), stop=(dt == NDT - 1))
                mean_n = rowpool.tile([1, S], MM, name="mean_n", tag="row")
                nc.vector.tensor_scalar(mean_n, ps_s, -1.0 / D, None, ALU.mult)
                var = rowpool.tile([1, S], F32, name="var", tag="row")
                nc.vector.tensor_scalar(var, ps_q, 1.0 / D, None, ALU.mult)
                m2 = rowpool.tile([1, S], F32, name="m2", tag="row")
                nc.vector.tensor_mul(m2, mean_n, mean_n)
                nc.vector.tensor_sub(var, var, m2)
                sd = rowpool.tile([1, S], F32, name="sd", tag="row")
                nc.scalar.activation(sd, var, AF.Sqrt, bias=eps_row)
                rs_r = rowpool.tile([1, S], MM, name="rs_r", tag="row")
                with nc.allow_low_precision(reason="f32r rows feed matmuls"):
                    nc.vector.reciprocal(rs_r, sd)
                rows_rs.append(rs_r)
                rows_nm.append(mean_n)
            for b in range(BPC):
                cs = slice(b * S, (b + 1) * S)
                pb_rs = psX.tile([P, S], F32, name="pb_rs", tag="px")
                nc.tensor.matmul(pb_rs, ones_row, rows_rs[b])
                rs_sb = tmppool.tile([P, S], F32, name="rs_sb", tag="tmp")
                nc.scalar.activation(rs_sb, pb_rs, AF.Copy)
                for dt in range(NDT):
                    pc = psX.tile([P, S], F32, name="pc", tag="px")
                    nc.tensor.matmul(pc, ident, src[dt][:, cs],
                                     start=True, stop=False)
                    nc.tensor.matmul(pc, ones_row, rows_nm[b],
                                     start=False, stop=True)
                    t1 = tmppool.tile([P, S], F32, name="t1", tag="tmp")
                    nc.vector.tensor_mul(t1, pc, rs_sb)
                    nc.scalar.activation(dst[dt][:, cs], t1, AF.Identity,
                                         scale=g_t[:, dt:dt + 1],
                                         bias=be_t[:, dt:dt + 1])

        def attn_phase(l):
            # ================= attention =================
            # ---- V (token-major, ones col per head) ----
            wv_sb = []
            for dt in range(NDT):
                wt = wpool.tile([P, D], MM, name=f"wv_{dt}", tag="w")
                nc.sync.dma_start(wt, wv_d[l, dt * P:(dt + 1) * P, :])
                wv_sb.append(wt)
            v_sb = []
            for tt in range(NDT):
                vt = vhpool.tile([P, H * VW], MM, name=f"v_{tt}", tag=f"vh_{tt}")
                nc.scalar.activation(
                    vt.rearrange("p (h e) -> p h e", e=VW)[:, :, DH:DH + 1],
                    onesH_f.rearrange("p (h e) -> p h e", e=1), AF.Copy)
                v_sb.append(vt)
            for tt in range(NDT):
                ts = slice(tt * P, (tt + 1) * P)
                for nh in range(2):
                    ps = psA.tile([P, S], F32, name="ps_v", tag="ps")
                    for dt in range(NDT):
                        nc.tensor.matmul(
                            ps, x_sb[dt][:, ts],
                            wv_sb[dt][:, nh * 512:(nh + 1) * 512],
                            start=(dt == 0), stop=False)
                    nc.tensor.matmul(
                        ps, ones_row[:, :P],
                        bv_sb[:, l * D + nh * 512: l * D + (nh + 1) * 512],
                        start=False, stop=True)
                    dstv = v_sb[tt][:, nh * 8 * VW:(nh + 1) * 8 * VW]
                    nc.scalar.activation(
                        dstv.rearrange("p (h e) -> p h e", e=VW)[:, :, 0:DH],
                        ps.rearrange("p (h e) -> p h e", e=DH),
                        AF.Copy)
            # ---- per head pair: Q, K, scores, softmax, AV ----
            # Software-pipelined with a one-stage skew: head i+1's
            # scores+exp are emitted before head i's AV/normalize, so the
            # in-order PE queue never stalls waiting on ACT(exp)/DVE(recip).
            o_sb = []
            pending = []

            def s2_flush():
                if pending:
                    pending.pop(0)()

            for hp in range(NHP):
                wq_p = wpool.tile([P, NDT, P], MM, name="wq_p", tag="w")
                nc.sync.dma_start(
                    wq_p, wq_d[l, hp].rearrange("(t p) m -> p t m", p=P))
                wk_p = wpool.tile([P, NDT, P], MM, name="wk_p", tag="w")
                nc.sync.dma_start(
                    wk_p, wk_d[l, hp].rearrange("(t p) m -> p t m", p=P))
                q_p = qw1pool.tile([P, NTOK], MM, name="q_p", tag="qw1")
                k_p = qw1pool.tile([P, NTOK], MM, name="k_p", tag="qw1")
                for b in range(BPC):
                    cs = slice(b * S, (b + 1) * S)
                    for wt, dst, bias in ((wq_p, q_p, bq_sb[l]),
                                          (wk_p, k_p, bk_sb[l])):
                        ps = psA.tile([P, S], F32, name="ps_qk", tag="ps")
                        for dt in range(NDT):
                            nc.tensor.matmul(ps, wt[:, dt, :],
                                             x_sb[dt][:, cs],
                                             start=(dt == 0),
                                             stop=(dt == NDT - 1))
                        nc.scalar.activation(dst[:, cs], ps, AF.Identity,
                                             bias=bias[:, hp:hp + 1])
                ot = oypool.tile([P, NTOK], MM, name=f"o_{hp}", tag=f"oy_{hp}")
                o_sb.append(ot)
                if KPART == "proj":
                    nc.scalar.activation(ot, q_p, AF.Copy)
                    continue
                for b in range(BPC):
                    cs = slice(b * S, (b + 1) * S)
                    for lh in range(2):
                        h = hp * 2 + lh
                        rb = lh * DH
                        rsl = slice(rb, rb + DH)
                        ets = []
                        for st in range(NST):
                            ps = psA.tile([P, S], F32, name="ps_sc", tag="ps")
                            nc.tensor.matmul(
                                ps,
                                k_p[rsl, b * S + st * P: b * S + (st + 1) * P],
                                q_p[rsl, cs])
                            et = epool.tile([P, S], MM, name="et", tag="e")
                            if KPART == "noexp":
                                nc.scalar.activation(et, ps, AF.Exp)
                            else:
                                nc.scalar.activation(
                                    et, ps, AF.Exp,
                                    scale=stau_sb[:, b:b + 1],
                                    bias=sdel_sb[:, b * NST + st:
                                                 b * NST + st + 1])
                            ets.append(et)

                        def s2(ets=ets, ot=ot, cs=cs, h=h, b=b, rsl=rsl):
                            po = psX.tile([VW, S], F32, name="po", tag="px")
                            for st in range(NST):
                                nc.tensor.matmul(
                                    po,
                                    v_sb[b * NST + st][:, h * VW:(h + 1) * VW],
                                    ets[st], start=(st == 0),
                                    stop=(st == NST - 1))
                            if KPART == "nosm":
                                nc.scalar.activation(ot[rsl, cs], po[:DH, :],
                                                     AF.Copy)
                                return
                            den_r = rowpool.tile([1, S], MM, name="den_r",
                                                 tag="row")
                            with nc.allow_low_precision(
                                    reason="f32r rows feed matmuls"):
                                nc.vector.reciprocal(den_r, po[DH:DH + 1, :])
                            pb = psX.tile([P, S], F32, name="pb_at", tag="px")
                            nc.tensor.matmul(pb[:DH, :], ones_row[:, :DH],
                                             den_r)
                            onum = tmppool.tile([P, S], F32, name="onum",
                                                tag="tmp")
                            nc.scalar.activation(onum[:DH, :], po[:DH, :],
                                                 AF.Copy)
                            nc.vector.tensor_mul(ot[rsl, cs], onum[:DH, :],
                                                 pb[:DH, :])

                        pending.append(s2)
                        if len(pending) > 1:
                            s2_flush()
            while pending:
                s2_flush()
            # ---- Wo projection + residual into x ----
            for dto in range(NDT):
                wo_p = wpool.tile([P, NDT, P], MM, name="wo_p", tag="w")
                nc.sync.dma_start(
                    wo_p, wo_d[l, dto].rearrange("(t p) m -> p t m", p=P))
                for b in range(BPC):
                    cs = slice(b * S, (b + 1) * S)
                    ps = psA.tile([P, S], F32, name="ps_wo", tag="ps")
                    for dt in range(NDT):
                        nc.tensor.matmul(ps, wo_p[:, dt, :], o_sb[dt][:, cs],
                                         start=(dt == 0), stop=(dt == NDT - 1))
                    t2 = tmppool.tile([P, S], F32, name="t2", tag="tmp")
                    nc.scalar.activation(t2, ps, AF.Identity,
                                         bias=bo_sb[l][:, dto:dto + 1])
                    nc.vector.tensor_add(x_sb[dto][:, cs], x_sb[dto][:, cs],
                                         t2)

        def ffn_phase(l):
            # ================= FFN =================
            y_sb = []
            for dt in range(NDT):
                yt = oypool.tile([P, NTOK], MM, name=f"y_{dt}", tag=f"oy_{dt}")
                y_sb.append(yt)
            # Software-pipelined chunks: chunk c+1's h-matmuls are emitted
            # before chunk c's y-matmuls (h tiles alternate tag groups) so
            # the PE never stalls on the gelu eviction tail.
            def h_block(fc):
                w1_sb = []
                for j in range(NDT // 2):
                    wt = qw1pool.tile([P, 2, FC], MM, name="w1c", tag="qw1")
                    nc.sync.dma_start(
                        wt, w1_d[l, fc, 2 * j:2 * j + 2].rearrange(
                            "d p f -> p d f"))
                    w1_sb.append(wt)
                h_sb = []
                for ft in range(FC // P):
                    ht = vhpool.tile([P, NTOK], MM, name="htile",
                                     tag=f"vh_{(fc % 2) * 4 + ft}")
                    h_sb.append(ht)
                    gfi = fc * (FC // P) + ft
                    for b in range(BPC):
                        cs = slice(b * S, (b + 1) * S)
                        ps = psA.tile([P, S], F32, name="ps_h", tag="ps")
                        for dt in range(NDT):
                            nc.tensor.matmul(
                                ps,
                                w1_sb[dt // 2][:, dt % 2,
                                               ft * P:(ft + 1) * P],
                                x_sb[dt][:, cs],
                                start=(dt == 0), stop=(dt == NDT - 1))
                        gelu_f = AF.Gelu if KGELU == "gelu" else AF.Identity
                        nc.scalar.activation(ht[:, cs], ps, gelu_f,
                                             bias=b1_sb[l][:, gfi:gfi + 1])
                return h_sb

            def y_block(fc, h_sb):
                w2_sb = []
                for i in range(FC // P):
                    wt = wpool.tile([P, D], MM, name="w2c", tag="w")
                    r0 = fc * FC + i * P
                    nc.sync.dma_start(wt, w2_d[l, r0:r0 + P, :])
                    w2_sb.append(wt)
                for dt in range(NDT):
                    for b in range(BPC):
                        cs = slice(b * S, (b + 1) * S)
                        ps = psA.tile([P, S], F32, name="ps_y", tag="ps")
                        for ft in range(FC // P):
                            nc.tensor.matmul(
                                ps, w2_sb[ft][:, dt * P:(dt + 1) * P],
                                h_sb[ft][:, cs],
                                start=(ft == 0), stop=(ft == FC // P - 1))
                        if fc == 0:
                            nc.scalar.activation(y_sb[dt][:, cs], ps,
                                                 AF.Identity,
                                                 bias=b2_sb[l][:, dt:dt + 1])
                        else:
                            nc.vector.tensor_add(y_sb[dt][:, cs], ps,
                                                 y_sb[dt][:, cs])

            prev = None
            for fc in range(NFC):
                h_now = h_block(fc)
                if prev is not None:
                    y_block(fc - 1, prev)
                prev = h_now
            y_block(NFC - 1, prev)
            # residual r2 = x_ln1 + y (into y); LN2(y) -> x
            for dt in range(NDT):
                nc.vector.tensor_add(y_sb[dt], y_sb[dt], x_sb[dt])
            if KPART == "full":
                ln(y_sb, x_sb, g2_sb[l], be2_sb[l])

        ATTN_LIKE = ("full", "attn", "noln", "proj", "noexp", "nosm")
        for l in range(L):
            if KPART in ATTN_LIKE:
                attn_phase(l)
            if KPART == "full":
                ln(x_sb, x_sb, g1_sb[l], be1_sb[l])
            if KPART in ("full", "ffn", "noln"):
                ffn_phase(l)

        # ---- final LN + Wp ----
        if KPART == "full":
            ln(x_sb, x_sb, gf_sb, bf_sb)
        for dto in range(NDT):
            wp_p = wpool.tile([P, NDT, P], MM, name="wp_p", tag="w")
            nc.sync.dma_start(wp_p,
                              wp_d[dto].rearrange("(t p) m -> p t m", p=P))
            for b in range(BPC):
                cs = slice(b * S, (b + 1) * S)
                ps = psA.tile([P, S], F32, name="ps_wp", tag="ps")
                for dt in range(NDT):
                    nc.tensor.matmul(ps, wp_p[:, dt, :], x_sb[dt][:, cs],
                                     start=(dt == 0), stop=(dt == NDT - 1))
                op = tmppool.tile([P, S], F32, name="outp", tag="tmp")
                nc.scalar.activation(op, ps, AF.Identity,
                                     bias=bp_sb[:, dto:dto + 1])
                nc.sync.dma_start(out_d[b, dto * P:(dto + 1) * P, :], op)

    if reps == 1:
        body()
    else:
        with tc.For_i(0, reps, 1) as i:
            body(i)
    ctx.close()


# ======================= host side =======================

def _prep_core_inputs(inputs):
    """Build the 8 per-core input maps (weights shared, x/tau/delta sharded)."""
    f = np.float32
    x = np.asarray(inputs["x"], f)
    tau = np.asarray(inputs["tau"], f)
    delta = np.asarray(inputs["delta"], f)
    scale = 1.0 / np.sqrt(np.float32(DH))

    shared = {}
    wq = np.asarray(inputs["Wq"], f)
    wk = np.asarray(inputs["Wk"], f)
    wv = np.asarray(inputs["Wv"], f)
    wo = np.asarray(inputs["Wo"], f)
    w1 = np.asarray(inputs["W1"], f)
    w2 = np.asarray(inputs["W2"], f)
    wp = np.asarray(inputs["Wp"], f)

    def col_tiled(wt):  # [L, D, D] (already [din, dout]) -> [L, NDT, D, P]
        return np.ascontiguousarray(
            wt.reshape(L, D, NDT, P).transpose(0, 2, 1, 3))

    wq_t = col_tiled(wq.transpose(0, 2, 1))
    wk_t = col_tiled(wk.transpose(0, 2, 1))
    wo_t = col_tiled(wo.transpose(0, 2, 1))
    wv_t = np.ascontiguousarray(wv.transpose(0, 2, 1))
    # W1 [L, F, D] -> W1^T [L, D, F] -> [L, NFC, NDT, P, FC]
    w1_tr = w1.transpose(0, 2, 1)
    w1_t = np.ascontiguousarray(
        w1_tr.reshape(L, NDT, P, NFC, FC).transpose(0, 3, 1, 2, 4))
    w2_t = np.ascontiguousarray(w2.transpose(0, 2, 1))  # [L, F, D]
    wp_t = np.ascontiguousarray(
        wp.transpose(1, 0).reshape(D, NDT, P).transpose(1, 0, 2))

    shared = {
        "wq_t": wq_t.astype(MMNP), "wk_t": wk_t.astype(MMNP),
        "wv_t": wv_t.astype(MMNP), "wo_t": wo_t.astype(MMNP),
        "w1_t": w1_t.astype(MMNP), "w2_t": w2_t.astype(MMNP),
        "wp_t": wp_t.astype(MMNP),
        "bq": np.asarray(inputs["bq"], f), "bk": np.asarray(inputs["bk"], f),
        "bv": np.asarray(inputs["bv"], f).astype(MMNP),
        "bo": np.asarray(inputs["bo"], f),
        "b1": np.asarray(inputs["b1"], f), "b2": np.asarray(inputs["b2"], f),
        "g1": np.asarray(inputs["g1"], f), "be1": np.asarray(inputs["be1"], f),
        "g2": np.asarray(inputs["g2"], f), "be2": np.asarray(inputs["be2"], f),
        "gf": np.asarray(inputs["gf"], f), "bf": np.asarray(inputs["bf"], f),
        "bp": np.asarray(inputs["bp"], f),
    }

    shared["ident"] = np.eye(P, dtype=f).astype(MMNP)

    in_maps = []
    for c in range(NCORES):
        bs = slice(c * BPC, (c + 1) * BPC)
        m = dict(shared)
        m["x_fm"] = np.ascontiguousarray(x[bs].transpose(0, 2, 1)).astype(MMNP)
        m["sc_tau"] = np.tile(tau[bs] * scale, (1, P)).astype(f)
        m["sc_delta"] = (delta[bs] * scale).astype(f)
        in_maps.append(m)
    return in_maps


def run(inputs, reps=1):
    nc = _build(reps)
    in_maps = _prep_core_inputs(inputs)
    res = bass_utils.run_bass_kernel_spmd(nc, in_maps,
                                          core_ids=list(range(NCORES)))
    outs = [res.results[c]["out_fm"].transpose(0, 2, 1) for c in range(NCORES)]
    return np.ascontiguousarray(np.concatenate(outs, axis=0))


def kernel(**inputs) -> np.ndarray:
    return run(inputs, reps=1)



# revision 35
# speedup vs baseline: 1.3003x; 1.1891x over previous
"""Trainium2 Bass kernel for a 2-layer de-stationary-attention transformer.

Model (per reference):
  L=2 layers of: x += DSAttn(x); x = LN1(x); x = LN2(x + FFN(x)); then
  final LN + output projection Wp.
  DSAttn: softmax(scale * (Q K^T * tau + delta)) V with per-batch tau,
  per-(batch, key) delta.

Shapes: B=16, S=512, D=1024, H=16 heads (dh=64), F=4096.

Sharding: data-parallel over batch across 8 NeuronCores (2 batches/core),
weights replicated. No collectives.

Per-core layout strategy (all on-chip between DRAM load and store):
  - Residual stream x kept FEATURE-major: 8 SBUF tiles [128 (d), 1024 (tok)].
    All matmuls contract over the partition dim, so projections consume x
    directly as the moving operand.
  - Scores are computed pre-transposed S^T[s, l] = K_slice^T Q_slice so the
    softmax denominator direction lands on the free dim of the AV matmul,
    and tau/delta fold into the ScalarE exp as scale/bias (both
    per-partition after the transpose).
  - V is produced token-major [tok, dout] (x as the stationary operand) with
    a ones-column appended per head, so each AV matmul also emits the
    softmax denominator row for free.
  - Per-token (free-dim) normalizers (softmax recip, LN mean/rstd) are
    broadcast across partitions with K=1 matmuls into PSUM.
  - LayerNorm column sums via ones-vector matmuls (partition reduction on
    the PE), then a 3-op normalize (2 DVE + 1 ACT with per-partition
    gain/bias).
  - Matmuls run in float32r (reduced-precision fp32, full PE rate at
    N>=256), accumulating in fp32 PSUM.

Weights are pre-transposed/pre-tiled on the host so every DMA is a large
contiguous read. Host also pre-transposes x to feature-major and folds the
1/sqrt(dh) softmax scale into tau/delta.
"""

import sys

if "/opt/trn_rl_repo" not in sys.path:
    sys.path.insert(0, "/opt/trn_rl_repo")

import numpy as np

import concourse.bass as bass
import concourse.bacc as bacc
import concourse.tile as tile
import concourse.mybir as mybir
from concourse import bass_utils

# Model dims
L, D, H, F = 2, 1024, 16, 4096
B, S = 16, 512
DH = D // H  # 64
NCORES = 8
BPC = B // NCORES  # batches per core
P = 128
NDT = D // P        # 8 d-tiles
NST = S // P        # 4 s-tiles per batch
NTOK = BPC * S      # 1024 tokens per core
NHP = H // 2        # 8 head pairs
FC = 512            # FFN f-chunk size
NFC = F // FC       # 8 chunks
VW = DH + 1         # 65: value width per head incl. ones column
EPS = 1e-5

F32 = mybir.dt.float32
import os as _os
MM = getattr(mybir.dt, _os.environ.get("MMDT", "bfloat16"))  # matmul operand dtype
import ml_dtypes
MMNP = np.dtype(mybir.dt.np(MM))
AF = mybir.ActivationFunctionType
ALU = mybir.AluOpType

_CACHE: dict = {}
import os
KPART = os.environ.get("KPART", "full")  # full | attn | ffn | noln
KGELU = os.environ.get("KGELU", "gelu")  # CoreSim lacks Gelu; "id" to swap


def _build(reps: int):
    key = (reps, KPART, KGELU)
    if key in _CACHE:
        return _CACHE[key]

    nc = bacc.Bacc("TRN2", target_bir_lowering=False, debug=False,
                   num_devices=NCORES)

    # ---- DRAM tensors (per-core shapes) ----
    # matmul-feeding tensors are float32r (same bits as f32)
    x_d = nc.dram_tensor("x_fm", (BPC, D, S), MM, kind="ExternalInput")
    wq_d = nc.dram_tensor("wq_t", (L, NDT, D, P), MM, kind="ExternalInput")
    wk_d = nc.dram_tensor("wk_t", (L, NDT, D, P), MM, kind="ExternalInput")
    wv_d = nc.dram_tensor("wv_t", (L, D, D), MM, kind="ExternalInput")
    wo_d = nc.dram_tensor("wo_t", (L, NDT, D, P), MM, kind="ExternalInput")
    w1_d = nc.dram_tensor("w1_t", (L, NFC, NDT, P, FC), MM, kind="ExternalInput")
    w2_d = nc.dram_tensor("w2_t", (L, F, D), MM, kind="ExternalInput")
    wp_d = nc.dram_tensor("wp_t", (NDT, D, P), MM, kind="ExternalInput")

    bq_d = nc.dram_tensor("bq", (L, D), F32, kind="ExternalInput")
    bk_d = nc.dram_tensor("bk", (L, D), F32, kind="ExternalInput")
    bo_d = nc.dram_tensor("bo", (L, D), F32, kind="ExternalInput")
    b1_d = nc.dram_tensor("b1", (L, F), F32, kind="ExternalInput")
    b2_d = nc.dram_tensor("b2", (L, D), F32, kind="ExternalInput")
    g1_d = nc.dram_tensor("g1", (L, D), F32, kind="ExternalInput")
    be1_d = nc.dram_tensor("be1", (L, D), F32, kind="ExternalInput")
    g2_d = nc.dram_tensor("g2", (L, D), F32, kind="ExternalInput")
    be2_d = nc.dram_tensor("be2", (L, D), F32, kind="ExternalInput")
    gf_d = nc.dram_tensor("gf", (D,), F32, kind="ExternalInput")
    bf_d = nc.dram_tensor("bf", (D,), F32, kind="ExternalInput")
    bp_d = nc.dram_tensor("bp", (D,), F32, kind="ExternalInput")
    ident_d = nc.dram_tensor("ident", (P, P), MM, kind="ExternalInput")
    stau_d = nc.dram_tensor("sc_tau", (BPC, P), F32, kind="ExternalInput")
    sdel_d = nc.dram_tensor("sc_delta", (BPC, S), F32, kind="ExternalInput")

    out_d = nc.dram_tensor("out_fm", (BPC, D, S), F32, kind="ExternalOutput")

    with tile.TileContext(nc) as tc:
        _emit(nc, tc, reps, locals())

    nc.compile()
    _CACHE[key] = nc
    return nc


def _emit(nc, tc, reps, d):
    x_d, wq_d, wk_d, wv_d, wo_d, w1_d, w2_d, wp_d = (
        d["x_d"], d["wq_d"], d["wk_d"], d["wv_d"], d["wo_d"], d["w1_d"],
        d["w2_d"], d["wp_d"])
    bq_d, bk_d, bo_d, b1_d, b2_d = (
        d["bq_d"], d["bk_d"], d["bo_d"], d["b1_d"], d["b2_d"])
    g1_d, be1_d, g2_d, be2_d, gf_d, bf_d, bp_d = (
        d["g1_d"], d["be1_d"], d["g2_d"], d["be2_d"], d["gf_d"], d["bf_d"],
        d["bp_d"])
    stau_d, sdel_d, out_d = d["stau_d"], d["sdel_d"], d["out_d"]
    ident_d = d["ident_d"]

    from contextlib import ExitStack
    ctx = ExitStack()
    # Static SBUF budget is ~192KB/partition; non-overlapping-lifetime
    # buffers share pool tags (o/y, v/h, qk/w1, out/tmp).
    singles = ctx.enter_context(tc.tile_pool(name="singles", bufs=1))
    xpool = ctx.enter_context(tc.tile_pool(name="xpool", bufs=1))
    vhpool = ctx.enter_context(tc.tile_pool(name="vhpool", bufs=1))
    oypool = ctx.enter_context(tc.tile_pool(name="oypool", bufs=1))
    qw1pool = ctx.enter_context(tc.tile_pool(name="qw1pool", bufs=5))
    wpool = ctx.enter_context(tc.tile_pool(name="wpool", bufs=8))
    epool = ctx.enter_context(tc.tile_pool(name="epool", bufs=16))
    tmppool = ctx.enter_context(tc.tile_pool(name="tmppool", bufs=4))
    rowpool = ctx.enter_context(tc.tile_pool(name="rowpool", bufs=5))
    psA = ctx.enter_context(tc.tile_pool(name="psA", bufs=4, space="PSUM"))
    psX = ctx.enter_context(tc.tile_pool(name="psX", bufs=4, space="PSUM"))

    # ---- constants / params (loaded once, outside the reps loop) ----
    # memset cannot write float32r; bounce constants through f32 + ACT copy
    ones_col_f = singles.tile([P, 1], F32)
    nc.vector.memset(ones_col_f, 1.0)
    ones_col = singles.tile([P, 1], MM)
    nc.scalar.activation(ones_col, ones_col_f, AF.Copy)
    ones_row_f = singles.tile([1, P], F32)
    nc.vector.memset(ones_row_f, 1.0)
    ones_row = singles.tile([1, P], MM)
    nc.scalar.activation(ones_row, ones_row_f, AF.Copy)
    onesH_f = singles.tile([P, H], F32)
    nc.vector.memset(onesH_f, 1.0)
    eps_row = singles.tile([1, 1], F32)
    nc.vector.memset(eps_row, EPS)


    def load_cols(dram_row, ncols):
        # [ncols*P] DRAM vector -> [P, ncols] SBUF (partition-major)
        t = singles.tile([P, ncols], dram_row.dtype,
                         name=f"prm_{dram_row.tensor.name}_{nc.next_id()}")
        nc.sync.dma_start(t, dram_row.rearrange("(t p) -> p t", p=P))
        return t

    bq_sb, bk_sb, bo_sb, b2_sb = [], [], [], []
    g1_sb, be1_sb, g2_sb, be2_sb, b1_sb = [], [], [], [], []
    for l in range(L):
        bq_sb.append(load_cols(bq_d[l], NDT))
        bk_sb.append(load_cols(bk_d[l], NDT))
        bo_sb.append(load_cols(bo_d[l], NDT))
        b2_sb.append(load_cols(b2_d[l], NDT))
        g1_sb.append(load_cols(g1_d[l], NDT))
        be1_sb.append(load_cols(be1_d[l], NDT))
        g2_sb.append(load_cols(g2_d[l], NDT))
        be2_sb.append(load_cols(be2_d[l], NDT))
        b1_sb.append(load_cols(b1_d[l], F // P))
    gf_sb = load_cols(gf_d.ap(), NDT)
    bf_sb = load_cols(bf_d.ap(), NDT)
    bp_sb = load_cols(bp_d.ap(), NDT)
    stau_sb = singles.tile([P, BPC], F32)
    nc.sync.dma_start(stau_sb, stau_d.ap().rearrange("b p -> p b"))
    sdel_sb = singles.tile([P, BPC * NST], F32)
    nc.sync.dma_start(sdel_sb.rearrange("p (b t) -> p b t", b=BPC),
                      sdel_d.ap().rearrange("b (t p) -> p b t", p=P))

    def body(_i=None):
        # ---- load x (feature-major) ----
        x_sb = []
        for dt in range(NDT):
            xt = xpool.tile([P, NTOK], MM, name=f"x_{dt}", tag=f"x_{dt}")
            for b in range(BPC):
                nc.sync.dma_start(
                    xt[:, b * S:(b + 1) * S],
                    x_d[b, dt * P:(dt + 1) * P, :])
            x_sb.append(xt)

        def ln_stats_alloc():
            """Per-batch (colsum, sq-colsum) PSUM accumulators."""
            return {b: (psX.tile([1, S], F32, name=f"lns{b}", tag="px"),
                        psX.tile([1, S], F32, name=f"lnq{b}", tag="px"))
                    for b in range(BPC)}

        def ln_stats_mm(st, src_tile, dt, b):
            """Emit colsum + sq-colsum matmuls for one (dt, b) tile slice.
            Called interleaved inside the producing GEMM loop so the PE
            stays busy during LN stat accumulation."""
            cs = slice(b * S, (b + 1) * S)
            ps_s, ps_q = st[b]
            nc.tensor.matmul(ps_s, ones_col, src_tile[:, cs],
                             start=(dt == 0), stop=(dt == NDT - 1))
            sq = tmppool.tile([P, S], MM, name="sq", tag="tmp")
            nc.scalar.activation(sq, src_tile[:, cs], AF.Square)
            nc.tensor.matmul(ps_q, ones_col, sq,
                             start=(dt == 0), stop=(dt == NDT - 1))

        def ln_finish(st, src, dst, g_t, be_t, stats_next=None):
            """Row chains (DVE) + rs/-m*rs broadcasts (2 K=1 matmuls per
            batch) + per-tile normalize: DVE mul, DVE add, ACT affine.
            If stats_next is given, the next LN's stat matmuls are emitted
            right after each normalized tile (keeps PE fed)."""
            rows_rs, rows_nm = [], []
            for b in range(BPC):
                ps_s, ps_q = st[b]
                mean_n = rowpool.tile([1, S], F32, name="mean_n", tag="row")
                nc.vector.tensor_scalar(mean_n, ps_s, -1.0 / D, None, ALU.mult)
                var = rowpool.tile([1, S], F32, name="var", tag="row")
                nc.vector.tensor_scalar(var, ps_q, 1.0 / D, None, ALU.mult)
                m2 = rowpool.tile([1, S], F32, name="m2", tag="row")
                nc.vector.tensor_mul(m2, mean_n, mean_n)
                nc.vector.tensor_sub(var, var, m2)
                sd = rowpool.tile([1, S], F32, name="sd", tag="row")
                nc.scalar.activation(sd, var, AF.Sqrt, bias=eps_row)
                rs_f = rowpool.tile([1, S], F32, name="rs_f", tag="row")
                nc.vector.reciprocal_approx_fast(rs_f, sd)
                rs_r = rowpool.tile([1, S], MM, name="rs_r", tag="row")
                nc.vector.tensor_scalar(rs_r, rs_f, 1.0, None, ALU.mult)
                nmrs = rowpool.tile([1, S], MM, name="nmrs", tag="row")
                nc.vector.tensor_mul(nmrs, mean_n, rs_f)
                rows_rs.append(rs_r)
                rows_nm.append(nmrs)
            for b in range(BPC):
                cs = slice(b * S, (b + 1) * S)
                pb_rs = psA.tile([P, S], F32, name="pb_rs", tag="ps")
                nc.tensor.matmul(pb_rs, ones_row, rows_rs[b])
                pb_nm = psA.tile([P, S], F32, name="pb_nm", tag="ps")
                nc.tensor.matmul(pb_nm, ones_row, rows_nm[b])
                for dt in range(NDT):
                    t1 = tmppool.tile([P, S], F32, name="t1", tag="tmp")
                    nc.vector.tensor_mul(t1, src[dt][:, cs], pb_rs)
                    t2 = tmppool.tile([P, S], F32, name="t2", tag="tmp")
                    nc.vector.tensor_add(t2, t1, pb_nm)
                    nc.scalar.activation(dst[dt][:, cs], t2, AF.Identity,
                                         scale=g_t[:, dt:dt + 1],
                                         bias=be_t[:, dt:dt + 1])
                    if stats_next is not None:
                        ln_stats_mm(stats_next, dst[dt], dt, b)

        def ln(src, dst, g_t, be_t, stats_next=None):
            st = ln_stats_alloc()
            for b in range(BPC):
                for dt in range(NDT):
                    ln_stats_mm(st, src[dt], dt, b)
            ln_finish(st, src, dst, g_t, be_t, stats_next)

        def attn_phase(l, want_stats=False):
            # ================= attention =================
            # ---- V (token-major, ones col per head) ----
            wv_sb = []
            for dt in range(NDT):
                wt = wpool.tile([P, D], MM, name=f"wv_{dt}", tag="w")
                nc.sync.dma_start(wt, wv_d[l, dt * P:(dt + 1) * P, :])
                wv_sb.append(wt)
            v_sb = []
            for tt in range(NDT):
                vt = vhpool.tile([P, H * VW], MM, name=f"v_{tt}", tag=f"vh_{tt}")
                nc.scalar.activation(
                    vt.rearrange("p (h e) -> p h e", e=VW)[:, :, DH:DH + 1],
                    onesH_f.rearrange("p (h e) -> p h e", e=1), AF.Copy)
                v_sb.append(vt)
            for tt in range(NDT):
                ts = slice(tt * P, (tt + 1) * P)
                for nh in range(2):
                    ps = psA.tile([P, S], F32, name="ps_v", tag="ps")
                    for dt in range(NDT):
                        nc.tensor.matmul(
                            ps, x_sb[dt][:, ts],
                            wv_sb[dt][:, nh * 512:(nh + 1) * 512],
                            start=(dt == 0), stop=(dt == NDT - 1))
                    dstv = v_sb[tt][:, nh * 8 * VW:(nh + 1) * 8 * VW]
                    nc.scalar.activation(
                        dstv.rearrange("p (h e) -> p h e", e=VW)[:, :, 0:DH],
                        ps.rearrange("p (h e) -> p h e", e=DH),
                        AF.Copy)
            # ---- per head pair: Q, K, scores, softmax, AV ----
            # Software-pipelined with a one-stage skew: head i+1's
            # scores+exp are emitted before head i's AV/normalize, so the
            # in-order PE queue never stalls waiting on ACT(exp)/DVE(recip).
            o_sb = []
            pending = []

            def s2_flush():
                if pending:
                    pending.pop(0)()

            for hp in range(NHP):
                wq_p = wpool.tile([P, NDT, P], MM, name="wq_p", tag="w")
                nc.sync.dma_start(
                    wq_p, wq_d[l, hp].rearrange("(t p) m -> p t m", p=P))
                wk_p = wpool.tile([P, NDT, P], MM, name="wk_p", tag="w")
                nc.sync.dma_start(
                    wk_p, wk_d[l, hp].rearrange("(t p) m -> p t m", p=P))
                q_p = qw1pool.tile([P, NTOK], MM, name="q_p", tag="qw1")
                k_p = qw1pool.tile([P, NTOK], MM, name="k_p", tag="qw1")
                for b in range(BPC):
                    cs = slice(b * S, (b + 1) * S)
                    for wt, dst, bias in ((wq_p, q_p, bq_sb[l]),
                                          (wk_p, k_p, bk_sb[l])):
                        ps = psA.tile([P, S], F32, name="ps_qk", tag="ps")
                        for dt in range(NDT):
                            nc.tensor.matmul(ps, wt[:, dt, :],
                                             x_sb[dt][:, cs],
                                             start=(dt == 0),
                                             stop=(dt == NDT - 1))
                        nc.scalar.activation(dst[:, cs], ps, AF.Identity,
                                             bias=bias[:, hp:hp + 1])
                ot = oypool.tile([P, NTOK], MM, name=f"o_{hp}", tag=f"oy_{hp}")
                o_sb.append(ot)
                if KPART == "proj":
                    nc.scalar.activation(ot, q_p, AF.Copy)
                    continue
                for b in range(BPC):
                    cs = slice(b * S, (b + 1) * S)
                    # scores for both heads of the pair: K=64 matmuls packed
                    # into upper/lower array row-halves via tile_position
                    ets = {0: [], 1: []}
                    for st in range(NST):
                        for lh in range(2):
                            rb = lh * DH
                            rsl = slice(rb, rb + DH)
                            ps = psA.tile([P, S], F32, name="ps_sc", tag="ps")
                            nc.tensor.matmul(
                                ps,
                                k_p[rsl, b * S + st * P: b * S + (st + 1) * P],
                                q_p[rsl, cs], tile_position=(rb, 0))
                            et = epool.tile([P, S], MM, name="et", tag="e")
                            if KPART == "noexp":
                                nc.scalar.activation(et, ps, AF.Exp)
                            else:
                                nc.scalar.activation(
                                    et, ps, AF.Exp,
                                    scale=stau_sb[:, b:b + 1],
                                    bias=sdel_sb[:, b * NST + st:
                                                 b * NST + st + 1])
                            ets[lh].append(et)

                    def s2(ets=ets, ot=ot, cs=cs, hp=hp, b=b):
                        po = {}
                        for lh in range(2):
                            h = hp * 2 + lh
                            po[lh] = psX.tile([VW, S], F32, name="po",
                                              tag="px")
                            for st in range(NST):
                                nc.tensor.matmul(
                                    po[lh],
                                    v_sb[b * NST + st][:, h * VW:(h + 1) * VW],
                                    ets[lh][st], start=(st == 0),
                                    stop=(st == NST - 1))
                        if KPART == "nosm":
                            for lh in range(2):
                                rsl = slice(lh * DH, (lh + 1) * DH)
                                nc.scalar.activation(ot[rsl, cs],
                                                     po[lh][:DH, :], AF.Copy)
                            return
        # paired denominators: one [1, 2S] row shared by both heads
                        den_c = rowpool.tile([1, 2 * S], F32, name="den_c",
                                             tag="row")
                        for lh in range(2):
                            nc.vector.tensor_scalar(
                                den_c[:, lh * S:(lh + 1) * S],
                                po[lh][DH:DH + 1, :], 1.0, None, ALU.mult)
                        den_f = rowpool.tile([1, 2 * S], F32, name="den_f",
                                             tag="row")
                        nc.vector.reciprocal_approx_fast(den_f, den_c)
                        den_r = rowpool.tile([1, 2 * S], MM, name="den_r",
                                             tag="row")
                        nc.vector.tensor_scalar(den_r, den_f, 1.0, None,
                                                ALU.mult)
                        for lh in range(2):
                            rsl = slice(lh * DH, (lh + 1) * DH)
                            pb = psX.tile([P, S], F32, name="pb_at", tag="px")
                            nc.tensor.matmul(
                                pb[:DH, :], ones_row[:, :DH],
                                den_r[:, lh * S:(lh + 1) * S])
                            onum = tmppool.tile([P, S], F32, name="onum",
                                                tag="tmp")
                            nc.scalar.activation(onum[:DH, :], po[lh][:DH, :],
                                                 AF.Copy)
                            nc.vector.tensor_mul(ot[rsl, cs], onum[:DH, :],
                                                 pb[:DH, :])

                    pending.append(s2)
                    if len(pending) > 1:
                        s2_flush()
            while pending:
                s2_flush()
            # ---- Wo projection + residual into x ----
            stats_out = None
            for dto in range(NDT):
                wo_p = wpool.tile([P, NDT, P], MM, name="wo_p", tag="w")
                nc.sync.dma_start(
                    wo_p, wo_d[l, dto].rearrange("(t p) m -> p t m", p=P))
                for b in range(BPC):
                    cs = slice(b * S, (b + 1) * S)
                    ps = psA.tile([P, S], F32, name="ps_wo", tag="ps")
                    for dt in range(NDT):
                        nc.tensor.matmul(ps, wo_p[:, dt, :], o_sb[dt][:, cs],
                                         start=(dt == 0), stop=(dt == NDT - 1))
                    t2 = tmppool.tile([P, S], F32, name="t2", tag="tmp")
                    nc.scalar.activation(t2, ps, AF.Identity,
                                         bias=bo_sb[l][:, dto:dto + 1])
                    nc.vector.tensor_add(x_sb[dto][:, cs], x_sb[dto][:, cs],
                                         t2)
                if stats_out is not None:
                    for b in range(BPC):
                        ln_stats_mm(stats_out, x_sb[dto], dto, b)
            return stats_out

        def ffn_phase(l, want_next_stats=False):
            # ================= FFN =================
            y_sb = []
            for dt in range(NDT):
                yt = oypool.tile([P, NTOK], MM, name=f"y_{dt}", tag=f"oy_{dt}")
                y_sb.append(yt)
            # Software-pipelined chunks: chunk c+1's h-matmuls are emitted
            # before chunk c's y-matmuls (h tiles alternate tag groups) so
            # the PE never stalls on the gelu eviction tail.
            def h_block(fc):
                w1_sb = []
                for j in range(NDT // 2):
                    wt = qw1pool.tile([P, 2, FC], MM, name="w1c", tag="qw1")
                    nc.sync.dma_start(
                        wt, w1_d[l, fc, 2 * j:2 * j + 2].rearrange(
                            "d p f -> p d f"))
                    w1_sb.append(wt)
                h_sb = []
                for ft in range(FC // P):
                    ht = vhpool.tile([P, NTOK], MM, name="htile",
                                     tag=f"vh_{(fc % 2) * 4 + ft}")
                    h_sb.append(ht)
                    gfi = fc * (FC // P) + ft
                    for b in range(BPC):
                        cs = slice(b * S, (b + 1) * S)
                        ps = psA.tile([P, S], F32, name="ps_h", tag="ps")
                        for dt in range(NDT):
                            nc.tensor.matmul(
                                ps,
                                w1_sb[dt // 2][:, dt % 2,
                                               ft * P:(ft + 1) * P],
                                x_sb[dt][:, cs],
                                start=(dt == 0), stop=(dt == NDT - 1))
                        gelu_f = AF.Gelu if KGELU == "gelu" else AF.Identity
                        nc.scalar.activation(ht[:, cs], ps, gelu_f,
                                             bias=b1_sb[l][:, gfi:gfi + 1])
                return h_sb

            def y_block(fc, h_sb, residual=False, stats_out=None):
                w2_sb = []
                for i in range(FC // P):
                    wt = wpool.tile([P, D], MM, name="w2c", tag="w")
                    r0 = fc * FC + i * P
                    nc.sync.dma_start(wt, w2_d[l, r0:r0 + P, :])
                    w2_sb.append(wt)
                for dt in range(NDT):
                    for b in range(BPC):
                        cs = slice(b * S, (b + 1) * S)
                        ps = psA.tile([P, S], F32, name="ps_y", tag="ps")
                        for ft in range(FC // P):
                            nc.tensor.matmul(
                                ps, w2_sb[ft][:, dt * P:(dt + 1) * P],
                                h_sb[ft][:, cs],
                                start=(ft == 0), stop=(ft == FC // P - 1))
                        if fc == 0:
                            nc.scalar.activation(y_sb[dt][:, cs], ps,
                                                 AF.Identity,
                                                 bias=b2_sb[l][:, dt:dt + 1])
                        else:
                            nc.vector.tensor_add(y_sb[dt][:, cs], ps,
                                                 y_sb[dt][:, cs])
                        if residual:
                            nc.vector.tensor_add(y_sb[dt][:, cs],
                                                 y_sb[dt][:, cs],
                                                 x_sb[dt][:, cs])
                        if stats_out is not None:
                            ln_stats_mm(stats_out, y_sb[dt], dt, b)

            prev = None
            for fc in range(NFC):
                h_now = h_block(fc)
                if prev is not None:
                    y_block(fc - 1, prev)
                prev = h_now
            y_block(NFC - 1, prev)
            # residual r2 = x_ln1 + y (into y); LN2(y) -> x
            for dt in range(NDT):
                nc.vector.tensor_add(y_sb[dt], y_sb[dt], x_sb[dt])
            if KPART == "full":
                ln(y_sb, x_sb, g2_sb[l], be2_sb[l])
            return None

        ATTN_LIKE = ("full", "attn", "noln", "proj", "noexp", "nosm")
        for l in range(L):
            if KPART in ATTN_LIKE:
                attn_phase(l)
            if KPART == "full":
                ln(x_sb, x_sb, g1_sb[l], be1_sb[l])
            if KPART in ("full", "ffn", "noln"):
                ffn_phase(l)

        # ---- final LN + Wp ----
        if KPART == "full":
            ln(x_sb, x_sb, gf_sb, bf_sb)
        for dto in range(NDT):
            wp_p = wpool.tile([P, NDT, P], MM, name="wp_p", tag="w")
            nc.sync.dma_start(wp_p,
                              wp_d[dto].rearrange("(t p) m -> p t m", p=P))
            for b in range(BPC):
                cs = slice(b * S, (b + 1) * S)
                ps = psA.tile([P, S], F32, name="ps_wp", tag="ps")
                for dt in range(NDT):
                    nc.tensor.matmul(ps, wp_p[:, dt, :], x_sb[dt][:, cs],
                                     start=(dt == 0), stop=(dt == NDT - 1))
                op = tmppool.tile([P, S], F32, name="outp", tag="tmp")
                nc.scalar.activation(op, ps, AF.Identity,
                                     bias=bp_sb[:, dto:dto + 1])
                nc.sync.dma_start(out_d[b, dto * P:(dto + 1) * P, :], op)

    if reps == 1:
        body()
    else:
        with tc.For_i(0, reps, 1) as i:
            body(i)
    ctx.close()


# ======================= host side =======================

def _prep_core_inputs(inputs):
    """Build the 8 per-core input maps (weights shared, x/tau/delta sharded)."""
    f = np.float32
    x = np.asarray(inputs["x"], f)
    tau = np.asarray(inputs["tau"], f)
    delta = np.asarray(inputs["delta"], f)
    scale = 1.0 / np.sqrt(np.float32(DH))

    shared = {}
    wq = np.asarray(inputs["Wq"], f)
    wk = np.asarray(inputs["Wk"], f)
    wv = np.asarray(inputs["Wv"], f)
    wo = np.asarray(inputs["Wo"], f)
    w1 = np.asarray(inputs["W1"], f)
    w2 = np.asarray(inputs["W2"], f)
    wp = np.asarray(inputs["Wp"], f)

    def col_tiled(wt):  # [L, D, D] (already [din, dout]) -> [L, NDT, D, P]
        return np.ascontiguousarray(
            wt.reshape(L, D, NDT, P).transpose(0, 2, 1, 3))

    wq_t = col_tiled(wq.transpose(0, 2, 1))
    wk_t = col_tiled(wk.transpose(0, 2, 1))
    wo_t = col_tiled(wo.transpose(0, 2, 1))
    wv_t = np.ascontiguousarray(wv.transpose(0, 2, 1))
    # W1 [L, F, D] -> W1^T [L, D, F] -> [L, NFC, NDT, P, FC]
    w1_tr = w1.transpose(0, 2, 1)
    w1_t = np.ascontiguousarray(
        w1_tr.reshape(L, NDT, P, NFC, FC).transpose(0, 3, 1, 2, 4))
    w2_t = np.ascontiguousarray(w2.transpose(0, 2, 1))  # [L, F, D]
    wp_t = np.ascontiguousarray(
        wp.transpose(1, 0).reshape(D, NDT, P).transpose(1, 0, 2))

    shared = {
        "wq_t": wq_t.astype(MMNP), "wk_t": wk_t.astype(MMNP),
        "wv_t": wv_t.astype(MMNP), "wo_t": wo_t.astype(MMNP),
        "w1_t": w1_t.astype(MMNP), "w2_t": w2_t.astype(MMNP),
        "wp_t": wp_t.astype(MMNP),
        "bq": np.asarray(inputs["bq"], f), "bk": np.asarray(inputs["bk"], f),
        "bo": (np.asarray(inputs["bo"], f)
               + np.einsum("ld,lod->lo", np.asarray(inputs["bv"], f),
                           np.asarray(inputs["Wo"], f))),
        "b1": np.asarray(inputs["b1"], f), "b2": np.asarray(inputs["b2"], f),
        "g1": np.asarray(inputs["g1"], f), "be1": np.asarray(inputs["be1"], f),
        "g2": np.asarray(inputs["g2"], f), "be2": np.asarray(inputs["be2"], f),
        "gf": np.asarray(inputs["gf"], f), "bf": np.asarray(inputs["bf"], f),
        "bp": np.asarray(inputs["bp"], f),
    }

    shared["ident"] = np.eye(P, dtype=f).astype(MMNP)

    in_maps = []
    for c in range(NCORES):
        bs = slice(c * BPC, (c + 1) * BPC)
        m = dict(shared)
        m["x_fm"] = np.ascontiguousarray(x[bs].transpose(0, 2, 1)).astype(MMNP)
        m["sc_tau"] = np.tile(tau[bs] * scale, (1, P)).astype(f)
        m["sc_delta"] = (delta[bs] * scale).astype(f)
        in_maps.append(m)
    return in_maps


def run(inputs, reps=1):
    nc = _build(reps)
    in_maps = _prep_core_inputs(inputs)
    res = bass_utils.run_bass_kernel_spmd(nc, in_maps,
                                          core_ids=list(range(NCORES)))
    outs = [res.results[c]["out_fm"].transpose(0, 2, 1) for c in range(NCORES)]
    return np.ascontiguousarray(np.concatenate(outs, axis=0))


def kernel(**inputs) -> np.ndarray:
    return run(inputs, reps=1)

